# revision 9
# baseline (speedup 1.0000x reference)
"""Decode-step multi-head attention with KV cache (DeepSpeed-inference style).

Full shapes (hardcoded per problem spec):
  query/key/value: [16, 1, 2048] f32
  key_cache/value_cache: [16, 16, 4096, 128] f32
  cache_len: scalar int (2048)
Output: [16, 1, 2048] f32

Strategy: data-parallel over batch across 8 NeuronCores (2 batches/core =
32 (batch, head) pairs per core). Per pair, the core streams the K and V
cache slices ([cache_len, 128] each) from HBM, computes scores with
multiply+reduce on VectorE (K stays in its natural [k, d] layout), exp via
ScalarE (with fused row-sum for the softmax denominator), and aggregates
V with TensorE matmuls (contraction over the k partition axis). The new
token's score/value is folded in as an extra column / extra matmul.

The kernel is bound by the 16 DMA engines' HBM read side (~24 GB/s per
engine on 8KB packets; K+V cache = 64MiB/core), so every engine packet is
a mandatory cache read and every other engine needs enough slack to never
stall the stream even on a core with slow engine clocks (observed ~25%
run-to-run engine-speed variance):

- K rides the gpsimd SWDGE queue cast f32->f16 in flight (free on the
  DMA-engine read side). 16-bit scores double VectorE throughput, the
  per-pair critical consumer.
- V rides the sync (SP) HWDGE queue in f32; the PE V-aggregation has
  ample slack for f32 weights.
- Pair 0 loads K+V on sync in f32: the SWDGE path starts ~3.5us later
  than HWDGE, and this keeps the DMA engines fed from the first moment.
- Small setup loads ride the scalar (ACT) HWDGE queue; queries are
  replicated across partitions with a PE outer product (ones x q_row),
  not a 2MB broadcast DMA.
- The last pair streams in halves whose tiles reuse the stream pool tags,
  so pool-slot dependencies pin those DMAs to the stream's end and the
  post-stream tail is half a pair's (16-bit) compute.
"""

import functools
import os
from contextlib import ExitStack

import numpy as np

import concourse.bacc as bacc
import concourse.bass as bass
import concourse.mybir as mybir
import concourse.tile as tile
from concourse import bass_utils

N_CORES = 8
P = 128  # partitions
NEG_BIG = -1e30

# test.py hooks: set TRACE=True before calling kernel() to collect a profile.
TRACE = False
TRACE_KWARGS = {}
LAST_RESULTS = None


def _build_program(bl: int, n_heads: int, max_seq: int, hd: int, cache_len: int):
    """Build + compile the per-core program. bl = local batch count."""
    npairs = bl * n_heads
    assert hd == P
    nch = cache_len // P          # full 128-row chunks of the cache
    rem = cache_len - nch * P     # remainder rows
    ncht = nch + (1 if rem else 0)
    sm_scale = 1.0 / float(np.sqrt(hd))
    # stream the last pair in halves so its compute overlaps its own DMA
    tail_halves = rem == 0 and nch % 2 == 0 and nch >= 4 and npairs >= 4
    nh = nch // 2 if tail_halves else 0

    nc = bacc.Bacc("TRN2", target_bir_lowering=False, debug=False)
    f32 = mybir.dt.float32
    f16 = mybir.dt.float16

    kc = nc.dram_tensor("kc", [bl, n_heads, max_seq, hd], f32, kind="ExternalInput").ap()
    vc = nc.dram_tensor("vc", [bl, n_heads, max_seq, hd], f32, kind="ExternalInput").ap()
    q = nc.dram_tensor("q", [npairs, hd], f32, kind="ExternalInput").ap()
    kn = nc.dram_tensor("kn", [npairs, hd], f32, kind="ExternalInput").ap()
    vn = nc.dram_tensor("vn", [npairs, hd], f32, kind="ExternalInput").ap()
    ident = nc.dram_tensor("ident", [P, P], f32, kind="ExternalInput").ap()
    out = nc.dram_tensor("out", [npairs, hd], f32, kind="ExternalOutput").ap()

    with tile.TileContext(nc) as tc, ExitStack() as ctx:
        singles = ctx.enter_context(tc.tile_pool(name="singles", bufs=1))
        kbufs = int(os.environ.get("KBUFS", "8"))
        kpool = ctx.enter_context(tc.tile_pool(name="kpool", bufs=kbufs))
        vpool = ctx.enter_context(tc.tile_pool(name="vpool", bufs=kbufs))
        ppool = ctx.enter_context(tc.tile_pool(name="ppool", bufs=3))
        stats = ctx.enter_context(tc.tile_pool(name="stats", bufs=6))
        psum_o = ctx.enter_context(tc.tile_pool(name="psum_o", bufs=2, space="PSUM"))
        psum_1 = ctx.enter_context(tc.tile_pool(name="psum_1", bufs=2, space="PSUM"))

        def emit_loads(b, h, first=False):
            vt = vpool.tile([P, ncht, hd], f32, tag="vt")
            if first:
                # pair 0 entirely on the sync HWDGE queue (f32): SWDGE
                # starts ~3.5us later, and this feeds the engines meanwhile
                kt = kpool.tile([P, ncht, hd], f32, tag="kt0", bufs=1)
                k_eng = nc.sync
            else:
                kt = kpool.tile([P, ncht, hd], f16, tag="kt")
                k_eng = nc.gpsimd
            if nch:
                kslc = kc[b, h, 0 : nch * P, :].rearrange("(p c) d -> p c d", c=nch)
                vslc = vc[b, h, 0 : nch * P, :].rearrange("(p c) d -> p c d", c=nch)
                k_eng.dma_start(out=kt[:, :nch, :], in_=kslc)
                nc.sync.dma_start(out=vt[:, :nch, :], in_=vslc)
            if rem:
                nc.vector.memset(vt[:, nch, :], 0.0)
                k_eng.dma_start(out=kt[:rem, nch, :], in_=kc[b, h, nch * P : cache_len, :])
                nc.sync.dma_start(out=vt[:rem, nch, :], in_=vc[b, h, nch * P : cache_len, :])
            return kt, vt

        # issue the first pairs' streaming loads before any setup traffic
        n_stream = npairs - 1 if tail_halves else npairs
        PRELOAD = min(3, n_stream)
        preloaded = [
            emit_loads(*divmod(p, n_heads), first=(p == 0)) for p in range(PRELOAD)
        ]

        ones_col = singles.tile([P, 1], f32)
        nc.vector.memset(ones_col, 1.0)

        # small setup loads ride the scalar (ACT) HWDGE queue so they never
        # delay the K/V streams
        q_row = singles.tile([1, npairs * hd], f32)
        q_row_src = bass.AP(
            tensor=q.tensor, offset=q.offset, ap=[[0, 1], [1, npairs * hd]]
        )
        nc.scalar.dma_start(out=q_row, in_=q_row_src)
        kn_all = singles.tile([npairs, hd], f32)
        nc.scalar.dma_start(out=kn_all, in_=kn)
        vn_all = singles.tile([npairs, hd], f32)
        nc.scalar.dma_start(out=vn_all, in_=vn)
        q_all = singles.tile([npairs, hd], f32)
        nc.scalar.dma_start(out=q_all, in_=q)
        ident_sb = singles.tile([P, P], f32)
        nc.scalar.dma_start(out=ident_sb, in_=ident)

        # all queries broadcast to every partition, once, as a PE outer
        # product ones[1,128] x q_row[1,*] (not DMA: a 2MB broadcast DMA
        # costs ~6.5us of DMA engine time; not gpsimd partition_broadcast:
        # that would serialize the Pool engine against K SWDGE generation).
        # Replicas are kept in f16 so the score multiply runs at 16-bit DVE
        # throughput; pair 0 additionally gets an f32 replica to match its
        # f32 K tile.
        ones_row = singles.tile([1, P], f32)
        nc.vector.memset(ones_row, 1.0)
        q_all_b = singles.tile([P, npairs, hd], f16)
        q0_b = singles.tile([P, hd], f32)
        GPAIRS = 4  # pairs per chunk; 4*hd f32 = one 2KB PSUM bank
        for g in range(npairs // GPAIRS):
            qb_ps = psum_1.tile([P, GPAIRS, hd], f32, tag="qb")
            qb_2d = bass.AP(
                tensor=qb_ps.tensor,
                offset=qb_ps.offset,
                ap=[qb_ps.ap[0], [1, GPAIRS * hd]],
            )
            nc.tensor.matmul(
                qb_2d,
                lhsT=ones_row,
                rhs=q_row[0:1, g * GPAIRS * hd : (g + 1) * GPAIRS * hd],
                start=True,
                stop=True,
            )
            nc.scalar.copy(q_all_b[:, g * GPAIRS : (g + 1) * GPAIRS, :], qb_ps)
            if g == 0:
                nc.scalar.copy(q0_b, qb_ps[:, 0, :])

        # Softmax denominators, one column per pair (partition 0).
        lrow = psum_1.tile([1, npairs], f32, tag="l")
        # Unnormalized cache-part outputs, head-dim on partitions, one
        # column per pair.
        out_sb = singles.tile([P, npairs], f32)

        # ---- new-token contribution, batched over all pairs ----
        prod_new = singles.tile([npairs, hd], f32)
        nc.vector.tensor_mul(prod_new, kn_all, q_all)
        s_new = singles.tile([npairs, 1], f32)
        nc.vector.reduce_sum(s_new, prod_new, axis=mybir.AxisListType.X)
        p_new = singles.tile([npairs, 1], f32)
        nc.scalar.activation(
            out=p_new, in_=s_new, func=mybir.ActivationFunctionType.Exp, scale=sm_scale
        )
        # rows 0..npairs-1: p_new[p] * v_new[p]; rest zero
        vns = singles.tile([P, hd], f32)
        nc.vector.memset(vns, 0.0)
        nc.vector.tensor_scalar_mul(vns[:npairs, :], vn_all, p_new)
        vnsT_ps = psum_1.tile([P, P], f32, tag="wide")
        nc.tensor.transpose(vnsT_ps, vns, ident_sb)
        vnsT = singles.tile([P, npairs], f32)
        nc.scalar.copy(vnsT, vnsT_ps[:, :npairs])

        def bcast(ap2d, nb):
            return bass.AP(
                tensor=ap2d.tensor,
                offset=ap2d.offset,
                ap=[ap2d.ap[0], [0, nb], ap2d.ap[1]],
            )

        def pair_compute(pr, kt, vt, blocks, prod_dtype, prod_tag):
            """blocks: list of (chunk_lo, chunk_hi). Returns nothing; writes
            lrow column pr and out_sb column pr."""
            q_b2 = q0_b if (pr == 0) else q_all_b[:, pr, :]
            nblk = len(blocks)
            lblk = stats.tile([P, nblk], f32, tag=f"lb{nblk}")
            acc = psum_o.tile([P, 1], f32, tag="acc")
            for bi, (lo, hi) in enumerate(blocks):
                nb = hi - lo
                prod = ppool.tile([P, nb, hd], prod_dtype, tag=prod_tag)
                s_blk = stats.tile([P, nb], f32, tag=f"s{prod_tag}")
                nc.vector.tensor_mul(prod, kt[:, lo:hi, :], bcast(q_b2, nb))
                nc.vector.reduce_sum(s_blk, prod, axis=mybir.AxisListType.X)
                p_blk = stats.tile([P, nb], f32, tag=f"p{prod_tag}")
                nc.scalar.activation(
                    out=p_blk,
                    in_=s_blk,
                    func=mybir.ActivationFunctionType.Exp,
                    scale=sm_scale,
                    accum_out=lblk[:, bi : bi + 1],
                )
                for c in range(nb):
                    nc.tensor.matmul(
                        acc,
                        lhsT=vt[:, lo + c, :],
                        rhs=p_blk[:, c : c + 1],
                        start=(bi == 0 and c == 0),
                        stop=(bi == nblk - 1 and c == nb - 1),
                    )
            if nblk == 1:
                l_part = lblk
            else:
                l_part = stats.tile([P, 1], f32, tag="l")
                nc.vector.reduce_sum(l_part, lblk, axis=mybir.AxisListType.X)
            nc.tensor.matmul(
                lrow[0:1, pr : pr + 1], lhsT=ones_col, rhs=l_part, start=True, stop=True
            )
            nc.scalar.copy(out_sb[:, pr : pr + 1], acc)

        assert rem == 0, "cache_len % 128 != 0 not needed for this problem"
        for p in range(n_stream):
            b, h = divmod(p, n_heads)
            kt, vt = preloaded[p] if p < len(preloaded) else emit_loads(b, h)
            pair_compute(
                p, kt, vt, [(0, ncht)], f32 if p == 0 else f16,
                "prod0" if p == 0 else "prod",
            )

        if tail_halves:
            # Last pair in halves. Its K/V tiles reuse the stream pool tags,
            # so pool-slot dependencies pin these DMAs behind the earlier
            # pairs' loads at the stream's end; each half's compute overlaps
            # the next half's DMA.
            p = npairs - 1
            b, h = divmod(p, n_heads)
            kt = kpool.tile([P, ncht, hd], f16, tag="kt")
            vt = vpool.tile([P, ncht, hd], f32, tag="vt")
            rows = nh * P
            for hi in range(2):
                r0 = hi * rows
                kslc = kc[b, h, r0 : r0 + rows, :].rearrange("(p c) d -> p c d", c=nh)
                vslc = vc[b, h, r0 : r0 + rows, :].rearrange("(p c) d -> p c d", c=nh)
                cs = slice(hi * nh, (hi + 1) * nh)
                nc.gpsimd.dma_start(out=kt[:, cs, :], in_=kslc)
                nc.sync.dma_start(out=vt[:, cs, :], in_=vslc)
            pair_compute(p, kt, vt, [(0, nh), (nh, 2 * nh)], f16, "prodh")

        # ---- epilogue: add new-token contribution, normalize, emit ----
        out_full = singles.tile([P, npairs], f32)
        nc.vector.tensor_add(out_full, out_sb, vnsT)

        # transpose the denominator row [1, npairs] -> [npairs, 1] with one
        # tiny PE matmul (lhsT = the row itself, rhs = a single one)
        lrow_sb = singles.tile([1, npairs], f32)
        nc.scalar.copy(lrow_sb, lrow)
        lT_ps = psum_1.tile([npairs, 1], f32, tag="l")
        nc.tensor.matmul(
            lT_ps, lhsT=lrow_sb, rhs=ones_col[0:1, 0:1], start=True, stop=True
        )
        l_tot = singles.tile([npairs, 1], f32)
        nc.vector.tensor_add(l_tot, lT_ps, p_new)
        recip_l = singles.tile([npairs, 1], f32)
        nc.vector.reciprocal(recip_l, l_tot)

        oT = psum_1.tile([npairs, hd], f32, tag="wide")
        nc.tensor.transpose(oT, out_full, ident_sb)

        final_sb = singles.tile([npairs, hd], f32)
        nc.scalar.mul(final_sb, oT, mul=recip_l)
        nc.scalar.dma_start(out=out, in_=final_sb)

    nc.compile()
    return nc


@functools.lru_cache(maxsize=4)
def _program(bl, n_heads, max_seq, hd, cache_len):
    return _build_program(bl, n_heads, max_seq, hd, cache_len)


def kernel(query, key, value, key_cache, value_cache, cache_len):
    global LAST_RESULTS
    query = np.asarray(query, dtype=np.float32)
    key = np.asarray(key, dtype=np.float32)
    value = np.asarray(value, dtype=np.float32)
    key_cache = np.asarray(key_cache, dtype=np.float32)
    value_cache = np.asarray(value_cache, dtype=np.float32)
    cache_len = int(cache_len)

    b_sz, q_len, d_model = query.shape
    _, n_heads, max_seq, hd = key_cache.shape
    assert q_len == 1 and d_model == n_heads * hd
    assert b_sz % N_CORES == 0
    bl = b_sz // N_CORES

    prog = _program(bl, n_heads, max_seq, hd, cache_len)

    ident = np.eye(P, dtype=np.float32)
    in_maps = []
    for i in range(N_CORES):
        sl = slice(i * bl, (i + 1) * bl)
        in_maps.append(
            {
                "kc": np.ascontiguousarray(key_cache[sl]),
                "vc": np.ascontiguousarray(value_cache[sl]),
                "q": np.ascontiguousarray(query[sl]).reshape(bl * n_heads, hd),
                "kn": np.ascontiguousarray(key[sl]).reshape(bl * n_heads, hd),
                "vn": np.ascontiguousarray(value[sl]).reshape(bl * n_heads, hd),
                "ident": ident,
            }
        )

    try:
        res = bass_utils.run_bass_kernel_spmd(
            prog, in_maps, core_ids=list(range(N_CORES)), trace=TRACE, **TRACE_KWARGS
        )
    except Exception:
        # A previously crashed NeuronCore can leave the first execution
        # attempt failing with a transient runtime error; retry once.
        res = bass_utils.run_bass_kernel_spmd(
            prog, in_maps, core_ids=list(range(N_CORES)), trace=TRACE, **TRACE_KWARGS
        )
    LAST_RESULTS = res
    outs = [res.results[i]["out"].reshape(bl, q_len, d_model) for i in range(N_CORES)]
    return np.concatenate(outs, axis=0)


# revision 10
# speedup vs baseline: 1.8304x; 1.8304x over previous
"""Decode-step multi-head attention with KV cache (DeepSpeed-inference style).

Full shapes (hardcoded per problem spec):
  query/key/value: [16, 1, 2048] f32
  key_cache/value_cache: [16, 16, 4096, 128] f32
  cache_len: scalar int (2048)
Output: [16, 1, 2048] f32

Strategy: data-parallel over batch across 8 NeuronCores (2 batches/core =
32 (batch, head) pairs per core). Per pair, the core streams the K and V
cache slices ([cache_len, 128] each) from HBM, computes scores with
multiply+reduce on VectorE (K stays in its natural [k, d] layout), exp via
ScalarE (with fused row-sum for the softmax denominator), and aggregates
V with TensorE matmuls (contraction over the k partition axis). The new
token's score/value is folded in as an extra column / extra matmul.

The kernel is bound by the 16 DMA engines' HBM read side (~24 GB/s per
engine on 8KB packets; K+V cache = 64MiB/core), so every engine packet is
a mandatory cache read and every compute engine needs enough slack never
to stall the stream, even on cores with slow engine clocks (observed ~25%
run-to-run engine-speed variance):

- K rides the sync (SP) HWDGE queue in f32; V rides the gpsimd SWDGE
  queue cast f32->f16 in flight (free on the DMA-engine read side, and PE
  f16 weights are ~10x faster to load than f32).
- ScalarE casts each K tile to f16 so the score multiply+reduce runs at
  16-bit VectorE throughput (~2.2us/pair vs ~4.6 in f32) - both engines
  end up with >2x slack against the DMA stream.
- Small setup loads ride the scalar (ACT) HWDGE queue; queries are
  replicated across partitions with a PE outer product (ones x q_row),
  not a 2MB broadcast DMA.
- The last pair streams in halves whose tiles reuse the stream pool tags,
  so pool-slot dependencies pin those DMAs to the stream's end; its
  compute skips the cast (f32 DVE straight from the f32 K tile) to keep
  the post-stream serial chain short.
"""

import functools
import os
from contextlib import ExitStack

import numpy as np

import concourse.bacc as bacc
import concourse.bass as bass
import concourse.mybir as mybir
import concourse.tile as tile
from concourse import bass_utils

N_CORES = 8
P = 128  # partitions

# test.py hooks: set TRACE=True before calling kernel() to collect a profile.
TRACE = False
TRACE_KWARGS = {}
LAST_RESULTS = None


def _build_program(bl: int, n_heads: int, max_seq: int, hd: int, cache_len: int):
    """Build + compile the per-core program. bl = local batch count."""
    npairs = bl * n_heads
    assert hd == P
    nch = cache_len // P          # full 128-row chunks of the cache
    rem = cache_len - nch * P     # remainder rows
    assert rem == 0, "cache_len % 128 != 0 not needed for this problem"
    ncht = nch
    sm_scale = 1.0 / float(np.sqrt(hd))
    # stream the last pair in halves so its compute overlaps its own DMA
    tail_halves = nch % 2 == 0 and nch >= 4 and npairs >= 4
    nh = nch // 2 if tail_halves else 0

    nc = bacc.Bacc("TRN2", target_bir_lowering=False, debug=False)
    f32 = mybir.dt.float32
    f16 = mybir.dt.float16

    kc = nc.dram_tensor("kc", [bl, n_heads, max_seq, hd], f32, kind="ExternalInput").ap()
    vc = nc.dram_tensor("vc", [bl, n_heads, max_seq, hd], f32, kind="ExternalInput").ap()
    q = nc.dram_tensor("q", [npairs, hd], f32, kind="ExternalInput").ap()
    kn = nc.dram_tensor("kn", [npairs, hd], f32, kind="ExternalInput").ap()
    vn = nc.dram_tensor("vn", [npairs, hd], f32, kind="ExternalInput").ap()
    ident = nc.dram_tensor("ident", [P, P], f32, kind="ExternalInput").ap()
    out = nc.dram_tensor("out", [npairs, hd], f32, kind="ExternalOutput").ap()

    with tile.TileContext(nc) as tc, ExitStack() as ctx:
        singles = ctx.enter_context(tc.tile_pool(name="singles", bufs=1))
        kbufs = int(os.environ.get("KBUFS", "8"))
        kpool = ctx.enter_context(tc.tile_pool(name="kpool", bufs=kbufs))
        vpool = ctx.enter_context(tc.tile_pool(name="vpool", bufs=kbufs))
        k16pool = ctx.enter_context(tc.tile_pool(name="k16pool", bufs=3))
        ppool = ctx.enter_context(tc.tile_pool(name="ppool", bufs=3))
        stats = ctx.enter_context(tc.tile_pool(name="stats", bufs=6))
        psum_o = ctx.enter_context(tc.tile_pool(name="psum_o", bufs=2, space="PSUM"))
        psum_1 = ctx.enter_context(tc.tile_pool(name="psum_1", bufs=2, space="PSUM"))

        def emit_loads(b, h):
            kt = kpool.tile([P, ncht, hd], f32, tag="kt")
            # V is cast to fp16 during the DMA (SWDGE): free on the DMA
            # engine read side, and PE loads f16 weights ~10x faster.
            vt = vpool.tile([P, ncht, hd], f16, tag="vt")
            kslc = kc[b, h, 0 : nch * P, :].rearrange("(p c) d -> p c d", c=nch)
            vslc = vc[b, h, 0 : nch * P, :].rearrange("(p c) d -> p c d", c=nch)
            nc.sync.dma_start(out=kt, in_=kslc)
            nc.gpsimd.dma_start(out=vt, in_=vslc)
            return kt, vt

        # issue the first pairs' streaming loads before any setup traffic so
        # the sync queue's first instruction is a K DMA and the gpsimd
        # queue's first work is V SWDGE generation
        n_stream = npairs - 1 if tail_halves else npairs
        PRELOAD = min(3, n_stream)
        preloaded = [emit_loads(*divmod(p, n_heads)) for p in range(PRELOAD)]

        ones_col = singles.tile([P, 1], f32)
        nc.vector.memset(ones_col, 1.0)

        # small setup loads ride the scalar (ACT) HWDGE queue so they never
        # delay the K/V streams
        q_row = singles.tile([1, npairs * hd], f32)
        q_row_src = bass.AP(
            tensor=q.tensor, offset=q.offset, ap=[[0, 1], [1, npairs * hd]]
        )
        nc.scalar.dma_start(out=q_row, in_=q_row_src)
        kn_all = singles.tile([npairs, hd], f32)
        nc.scalar.dma_start(out=kn_all, in_=kn)
        vn_all = singles.tile([npairs, hd], f32)
        nc.scalar.dma_start(out=vn_all, in_=vn)
        q_all = singles.tile([npairs, hd], f32)
        nc.scalar.dma_start(out=q_all, in_=q)
        ident_sb = singles.tile([P, P], f32)
        nc.scalar.dma_start(out=ident_sb, in_=ident)

        # all queries broadcast to every partition, once, as a PE outer
        # product ones[1,128] x q_row[1,*] (not DMA: a 2MB broadcast DMA
        # costs ~6.5us of DMA engine time; not gpsimd partition_broadcast:
        # that would serialize the Pool engine against V SWDGE generation).
        # f16 replicas feed the 16-bit score path; the tail pair gets an
        # f32 replica for its cast-free f32 path.
        ones_row = singles.tile([1, P], f32)
        nc.vector.memset(ones_row, 1.0)
        q_all_b = singles.tile([P, npairs, hd], f16)
        qT_b = singles.tile([P, hd], f32)
        GPAIRS = 4  # pairs per chunk; 4*hd f32 = one 2KB PSUM bank
        ngrp = npairs // GPAIRS
        for g in range(ngrp):
            qb_ps = psum_1.tile([P, GPAIRS, hd], f32, tag="qb")
            qb_2d = bass.AP(
                tensor=qb_ps.tensor,
                offset=qb_ps.offset,
                ap=[qb_ps.ap[0], [1, GPAIRS * hd]],
            )
            nc.tensor.matmul(
                qb_2d,
                lhsT=ones_row,
                rhs=q_row[0:1, g * GPAIRS * hd : (g + 1) * GPAIRS * hd],
                start=True,
                stop=True,
            )
            nc.scalar.copy(q_all_b[:, g * GPAIRS : (g + 1) * GPAIRS, :], qb_ps)
            if tail_halves and g == ngrp - 1:
                nc.scalar.copy(qT_b, qb_ps[:, GPAIRS - 1, :])

        # Softmax denominators, one column per pair (partition 0).
        lrow = psum_1.tile([1, npairs], f32, tag="l")
        # Unnormalized cache-part outputs, head-dim on partitions, one
        # column per pair.
        out_sb = singles.tile([P, npairs], f32)

        # ---- new-token contribution, batched over all pairs ----
        prod_new = singles.tile([npairs, hd], f32)
        nc.vector.tensor_mul(prod_new, kn_all, q_all)
        s_new = singles.tile([npairs, 1], f32)
        nc.vector.reduce_sum(s_new, prod_new, axis=mybir.AxisListType.X)
        p_new = singles.tile([npairs, 1], f32)
        nc.scalar.activation(
            out=p_new, in_=s_new, func=mybir.ActivationFunctionType.Exp, scale=sm_scale
        )
        # rows 0..npairs-1: p_new[p] * v_new[p]; rest zero
        vns = singles.tile([P, hd], f32)
        nc.vector.memset(vns, 0.0)
        nc.vector.tensor_scalar_mul(vns[:npairs, :], vn_all, p_new)
        vnsT_ps = psum_1.tile([P, P], f32, tag="wide")
        nc.tensor.transpose(vnsT_ps, vns, ident_sb)
        vnsT = singles.tile([P, npairs], f32)
        nc.scalar.copy(vnsT, vnsT_ps[:, :npairs])

        def bcast(ap2d, nb):
            return bass.AP(
                tensor=ap2d.tensor,
                offset=ap2d.offset,
                ap=[ap2d.ap[0], [0, nb], ap2d.ap[1]],
            )

        def pair_tail_ops(pr, lblk, nblk, acc):
            if nblk == 1:
                l_part = lblk
            else:
                l_part = stats.tile([P, 1], f32, tag="l")
                nc.vector.reduce_sum(l_part, lblk, axis=mybir.AxisListType.X)
            nc.tensor.matmul(
                lrow[0:1, pr : pr + 1], lhsT=ones_col, rhs=l_part, start=True, stop=True
            )
            nc.scalar.copy(out_sb[:, pr : pr + 1], acc)

        for p in range(n_stream):
            b, h = divmod(p, n_heads)
            kt, vt = preloaded[p] if p < len(preloaded) else emit_loads(b, h)

            # ScalarE casts K to f16 so DVE runs the multiply+reduce at
            # 16-bit (2x) throughput
            kt16 = k16pool.tile([P, ncht, hd], f16, tag="kt16")
            nc.scalar.copy(kt16, kt)

            prod = ppool.tile([P, ncht, hd], f16, tag="prod")
            s_tile = stats.tile([P, ncht], f32, tag="s")
            nc.vector.tensor_mul(prod, kt16, bcast(q_all_b[:, p, :], ncht))
            nc.vector.reduce_sum(s_tile, prod, axis=mybir.AxisListType.X)

            p_tile = stats.tile([P, ncht], f16, tag="p")
            l_part = stats.tile([P, 1], f32, tag="l")
            nc.scalar.activation(
                out=p_tile,
                in_=s_tile,
                func=mybir.ActivationFunctionType.Exp,
                scale=sm_scale,
                accum_out=l_part,
            )

            acc = psum_o.tile([P, 1], f32, tag="acc")
            for c in range(ncht):
                nc.tensor.matmul(
                    acc,
                    lhsT=vt[:, c, :],
                    rhs=p_tile[:, c : c + 1],
                    start=(c == 0),
                    stop=(c == ncht - 1),
                )
            pair_tail_ops(p, l_part, 1, acc)

        if tail_halves:
            # Last pair in halves. Its K/V tiles reuse the stream pool tags,
            # so pool-slot dependencies pin these DMAs behind the earlier
            # pairs' loads at the stream's end; each half's compute overlaps
            # the next half's DMA. The cast is skipped (f32 DVE) to keep the
            # post-stream serial chain short.
            p = npairs - 1
            b, h = divmod(p, n_heads)
            kt = kpool.tile([P, ncht, hd], f32, tag="kt")
            vt = vpool.tile([P, ncht, hd], f16, tag="vt")
            rows = nh * P
            for hi in range(2):
                r0 = hi * rows
                kslc = kc[b, h, r0 : r0 + rows, :].rearrange("(p c) d -> p c d", c=nh)
                vslc = vc[b, h, r0 : r0 + rows, :].rearrange("(p c) d -> p c d", c=nh)
                cs = slice(hi * nh, (hi + 1) * nh)
                nc.sync.dma_start(out=kt[:, cs, :], in_=kslc)
                nc.gpsimd.dma_start(out=vt[:, cs, :], in_=vslc)

            l2 = stats.tile([P, 2], f32, tag="l2")
            acc = psum_o.tile([P, 1], f32, tag="acc")
            for hi in range(2):
                cs = slice(hi * nh, (hi + 1) * nh)
                prodh = ppool.tile([P, nh, hd], f32, tag="prodh")
                s_h = stats.tile([P, nh], f32, tag="sh")
                nc.vector.tensor_mul(prodh, kt[:, cs, :], bcast(qT_b, nh))
                nc.vector.reduce_sum(s_h, prodh, axis=mybir.AxisListType.X)
                p_h = stats.tile([P, nh], f16, tag="ph")
                nc.scalar.activation(
                    out=p_h,
                    in_=s_h,
                    func=mybir.ActivationFunctionType.Exp,
                    scale=sm_scale,
                    accum_out=l2[:, hi : hi + 1],
                )
                for c in range(nh):
                    nc.tensor.matmul(
                        acc,
                        lhsT=vt[:, hi * nh + c, :],
                        rhs=p_h[:, c : c + 1],
                        start=(hi == 0 and c == 0),
                        stop=(hi == 1 and c == nh - 1),
                    )
            pair_tail_ops(p, l2, 2, acc)

        # ---- epilogue: add new-token contribution, normalize, emit ----
        out_full = singles.tile([P, npairs], f32)
        nc.vector.tensor_add(out_full, out_sb, vnsT)

        # transpose the denominator row [1, npairs] -> [npairs, 1] with one
        # tiny PE matmul (lhsT = the row itself, rhs = a single one)
        lrow_sb = singles.tile([1, npairs], f32)
        nc.scalar.copy(lrow_sb, lrow)
        lT_ps = psum_1.tile([npairs, 1], f32, tag="l")
        nc.tensor.matmul(
            lT_ps, lhsT=lrow_sb, rhs=ones_col[0:1, 0:1], start=True, stop=True
        )
        l_tot = singles.tile([npairs, 1], f32)
        nc.vector.tensor_add(l_tot, lT_ps, p_new)
        recip_l = singles.tile([npairs, 1], f32)
        nc.vector.reciprocal(recip_l, l_tot)

        oT = psum_1.tile([npairs, hd], f32, tag="wide")
        nc.tensor.transpose(oT, out_full, ident_sb)

        final_sb = singles.tile([npairs, hd], f32)
        nc.scalar.mul(final_sb, oT, mul=recip_l)
        nc.scalar.dma_start(out=out, in_=final_sb)

    nc.compile()
    return nc


@functools.lru_cache(maxsize=4)
def _program(bl, n_heads, max_seq, hd, cache_len):
    return _build_program(bl, n_heads, max_seq, hd, cache_len)


def kernel(query, key, value, key_cache, value_cache, cache_len):
    global LAST_RESULTS
    query = np.asarray(query, dtype=np.float32)
    key = np.asarray(key, dtype=np.float32)
    value = np.asarray(value, dtype=np.float32)
    key_cache = np.asarray(key_cache, dtype=np.float32)
    value_cache = np.asarray(value_cache, dtype=np.float32)
    cache_len = int(cache_len)

    b_sz, q_len, d_model = query.shape
    _, n_heads, max_seq, hd = key_cache.shape
    assert q_len == 1 and d_model == n_heads * hd
    assert b_sz % N_CORES == 0
    bl = b_sz // N_CORES

    prog = _program(bl, n_heads, max_seq, hd, cache_len)

    ident = np.eye(P, dtype=np.float32)
    in_maps = []
    for i in range(N_CORES):
        sl = slice(i * bl, (i + 1) * bl)
        in_maps.append(
            {
                "kc": np.ascontiguousarray(key_cache[sl]),
                "vc": np.ascontiguousarray(value_cache[sl]),
                "q": np.ascontiguousarray(query[sl]).reshape(bl * n_heads, hd),
                "kn": np.ascontiguousarray(key[sl]).reshape(bl * n_heads, hd),
                "vn": np.ascontiguousarray(value[sl]).reshape(bl * n_heads, hd),
                "ident": ident,
            }
        )

    try:
        res = bass_utils.run_bass_kernel_spmd(
            prog, in_maps, core_ids=list(range(N_CORES)), trace=TRACE, **TRACE_KWARGS
        )
    except Exception:
        # A previously crashed NeuronCore can leave the first execution
        # attempt failing with a transient runtime error; retry once.
        res = bass_utils.run_bass_kernel_spmd(
            prog, in_maps, core_ids=list(range(N_CORES)), trace=TRACE, **TRACE_KWARGS
        )
    LAST_RESULTS = res
    outs = [res.results[i]["out"].reshape(bl, q_len, d_model) for i in range(N_CORES)]
    return np.concatenate(outs, axis=0)
